# revision 12
# baseline (speedup 1.0000x reference)
"""2-layer GCN encoder as a distributed Bass kernel on 8 TRN2 NeuronCores.

Decomposition (per core, nodes sharded by destination):
  hs1[v]  = dinv[v] * (x[v] @ W1)                 (own rows, AllGather, bf16)
  S1T[:,d]= sum_{e: dst=d} hs1[src_e]             (dma_gather + one-hot matmul,
                                                   accumulated TRANSPOSED)
  hsrT    = relu(dinv_col * S1T + b1)             (dst-side dinv per column,
                                                   bias per partition)
  hsr2[v] = dinv[v] * (hsrT^T @ W2)               (W2 commutes with the layer-2
                                                   edge sum -> aggregate at 64)
  S2[d]   = sum_{e: dst=d} hsr2[src_e]            (AllGather + gather at 64 wide)
  y[d]    = dinv[d]*S2[d] + b2

The one-hot scatter matrices (0/1/multiplicity) are streamed once as fp8_e4m3
and kept SBUF-resident for both layers (mixed fp8 x bf16 matmul is exact for
these values).  All float normalization (dinv = rsqrt(deg)) is computed on
device.  Edge gathers use SWDGE dma_gather (<=1024 indices per instruction,
round-robin over 4 SWDGE queues).  Self-loops are extra identity matmuls on
the block's own SBUF-kept tiles.
"""

import numpy as np

import concourse.bass as bass
import concourse.bacc as bacc
import concourse.mybir as mybir
import concourse.tile as tile
from concourse import library_config
from concourse.bass_utils import run_bass_kernel_spmd

F32 = mybir.dt.float32
BF16 = mybir.dt.bfloat16
FP8 = mybir.dt.float8e4
I16 = mybir.dt.int16

NCORES = 8
BLK = 128
# 128-index chunks per dma_gather instruction: the SWDGE descriptor ring
# holds ~100 descriptors per DMA engine and a gather generates num_idxs/16
# per ring (>=1792 indices per instruction wedges the ring-reclaim wait).
# 6 chunks = 48 descriptors/ring lets TWO gathers queue per ring, so the
# SDMA engines drain back-to-back instead of idling through each
# emit->doorbell->drain->semaphore round trip.
MAXCH = 6
NQUEUES = 4


def _cdiv(a, b):
    return (a + b - 1) // b


def preprocess(x, edge_index, ncores=NCORES):
    """Host-side graph partitioning: shard nodes/edges by dst, sort edges,
    build per-core gather indices (SWDGE wrapped layout), the 0/1 one-hot
    chunk matrices (fp8), and per-node degree counts.  Index/structure work
    plus dtype casts only; all float math happens on device."""
    import ml_dtypes

    N, IN = x.shape
    assert N % ncores == 0
    NP = N // ncores
    nblk = _cdiv(NP, BLK)

    src = np.asarray(edge_index[0], dtype=np.int64)
    dst = np.asarray(edge_index[1], dtype=np.int64)
    # degree includes the self-loop, but self-loops are handled by an
    # identity matmul on device, not by the edge gather
    deg = (np.bincount(dst, minlength=N) + 1).astype(np.float32)

    # dedupe repeated (src, dst) pairs; multiplicity goes into the multi-hot
    key = dst * N + src
    ukey, mult = np.unique(key, return_counts=True)
    dst_s = ukey // N
    src_s = ukey % N

    bounds = np.array(
        [i * NP + b * BLK for i in range(ncores) for b in range(nblk)] + [N],
        dtype=np.int64,
    )
    pos = np.searchsorted(dst_s, bounds)

    # dedupe sources within each (core, block): the one-hot row scatters to
    # all of that source's dst columns, and gathering each unique src once
    # in ascending order improves DRAM locality.  The one-hot absorbs any
    # edge permutation.
    blk_usrc = {}
    blk_scatter = {}  # (p, col_within_block, value) triplets
    ucnt = np.zeros((ncores, nblk), np.int64)
    for i in range(ncores):
        for b in range(nblk):
            k = i * nblk + b
            s0, s1 = pos[k], pos[k + 1]
            usrc, inv = np.unique(src_s[s0:s1], return_inverse=True)
            ucnt[i, b] = len(usrc)
            dl = (dst_s[s0:s1] - (i * NP + b * BLK)).astype(np.int64)
            blk_usrc[i, b] = usrc
            blk_scatter[i, b] = (inv, dl, mult[s0:s1].astype(np.float32))

    # uniform chunk counts across cores (SPMD: one program for all cores)
    CH = np.maximum(1, _cdiv(ucnt.max(axis=0), 128)).astype(np.int64)
    cofs = np.concatenate([[0], np.cumsum(CH)]).astype(np.int64)
    NCHT = int(CH.sum())
    widths = [min(BLK, NP - b * BLK) for b in range(nblk)]

    # per-segment valid counts must be uniform across cores: num_idxs_reg is
    # baked into the shared program, and the ucode asserts it equals the
    # count of non-negative indices
    segs = []  # (block, seg_start_chunk, seg_chunks)
    for b in range(nblk):
        for s0 in range(0, int(CH[b]), MAXCH):
            segs.append((b, s0, min(MAXCH, int(CH[b]) - s0)))
    seg_valid = []
    for (b, s0, sch) in segs:
        lo = s0 * 128
        v = max(min(int(ucnt[i, b]) - lo, sch * 128) for i in range(ncores))
        seg_valid.append(max(v, 1))

    # split-AllGather row layout: both collectives are issued in two pieces
    # (rows [0, PA) and [PA, NP) of every rank) so the first piece's
    # transfer overlaps the producing phase; the full tables are
    # piece-major, each piece rank-major.
    PA = min(NP, (nblk // 2) * BLK)
    PB = NP - PA

    def remap(v):
        r = v // NP
        j = v % NP
        return np.where(j < PA, r * PA + j, ncores * PA + r * PB + (j - PA))

    # The SWDGE ucode assigns logical index position i to descriptor ring
    # i % 16 (one ring per SDMA engine).  Permute each segment's indices so
    # ring r gets a CONTIGUOUS ascending slice of the sorted sources: each
    # engine's descriptor stream then walks HBM monotonically instead of
    # jumping by 16 rows per descriptor.
    def ring_perm(nv):
        # logical position for sorted slot q in [0, nv)
        n_r = (nv - np.arange(16) + 15) // 16  # slots per ring
        off = np.concatenate([[0], np.cumsum(n_r)])
        i_of_q = np.empty(nv, np.int64)
        for r in range(16):
            j = np.arange(n_r[r])
            i_of_q[off[r] + j] = j * 16 + r
        return i_of_q

    seg_perm = {sv: ring_perm(sv) for sv in set(seg_valid)}

    per_core = []
    for i in range(ncores):
        gidx = np.full((128, NCHT * 8), -1, np.int16)
        ohs = np.zeros((128, NCHT * 128), np.float32)
        for b in range(nblk):
            usrc = blk_usrc[i, b]
            inv, dl, mlt = blk_scatter[i, b]
            ne = len(usrc)
            # logical position of each sorted slot within its block,
            # segment by segment
            slot_to_i = np.empty(int(CH[b]) * 128, np.int64)
            for si, (bb, s0, sch) in enumerate(segs):
                if bb != b:
                    continue
                lo = s0 * 128
                nv = seg_valid[si]
                perm = seg_perm[nv]
                slot_to_i[lo : lo + nv] = lo + perm
                slot_to_i[lo + nv : lo + sch * 128] = np.arange(lo + nv, lo + sch * 128)
                # this core's values for the segment: real sources then
                # index-0 padding up to nv, then -1 up to capacity
                nreal = min(max(ne - lo, 0), sch * 128)
                vals = np.full(sch * 128, -1, np.int64)
                vals[perm[:nreal]] = remap(usrc[lo : lo + nreal])
                vals[perm[nreal:nv]] = 0
                wr = vals.reshape(sch * 8, 16).T  # [16, sch*8]
                gidx[:, (cofs[b] + s0) * 8 : (cofs[b] + s0 + sch) * 8] = np.tile(
                    wr, (8, 1)
                )
            # multi-hot: the edge whose unique src sits at logical position
            # i adds its multiplicity at (partition i%128, chunk i//128,
            # col = local dst)
            iq = slot_to_i[inv]
            c = iq // 128
            p = iq % 128
            np.add.at(ohs, (p, (cofs[b] + c) * 128 + dl), mlt)
        degp = np.concatenate(
            [deg[i * NP : (i + 1) * NP], np.ones(nblk * BLK - NP, np.float32)]
        )
        per_core.append(
            {
                "x_tr": np.ascontiguousarray(x[i * NP : (i + 1) * NP].T),
                "deg_own": np.ascontiguousarray(degp.reshape(nblk, BLK).T),
                "deg_row": np.ascontiguousarray(degp.reshape(1, nblk * BLK)),
                "gidx": gidx,
                "ohs": ohs.astype(ml_dtypes.float8_e4m3),
            }
        )

    meta = {
        "PA": int(PA),
        "PB": int(PB),
        "seg_valid": seg_valid,
        "N": N,
        "NP": NP,
        "IN": IN,
        "nblk": nblk,
        "CH": [int(c) for c in CH],
        "cofs": [int(c) for c in cofs],
        "widths": widths,
        "NCHT": NCHT,
    }
    return per_core, meta


def build_nc(meta, HID, OUT, ncores=NCORES):
    N, NP, IN = meta["N"], meta["NP"], meta["IN"]
    nblk, CH, cofs, widths = meta["nblk"], meta["CH"], meta["cofs"], meta["widths"]
    seg_valid = meta["seg_valid"]
    NCHT = meta["NCHT"]
    KC = IN // 128
    assert IN % 128 == 0 and HID == 128 and OUT <= 512

    nc = bacc.Bacc(
        "TRN2",
        target_bir_lowering=False,
        debug=False,
        num_devices=ncores,
        num_swdge_queues=NQUEUES,
    )

    x_tr = nc.dram_tensor("x_tr", [IN, NP], F32, kind="ExternalInput")
    w1 = nc.dram_tensor("w1", [IN, HID], F32, kind="ExternalInput")
    b1c = nc.dram_tensor("b1c", [HID, 1], F32, kind="ExternalInput")
    w2 = nc.dram_tensor("w2", [HID, OUT], F32, kind="ExternalInput")
    b2 = nc.dram_tensor("b2", [1, OUT], F32, kind="ExternalInput")
    deg_own = nc.dram_tensor("deg_own", [128, nblk], F32, kind="ExternalInput")
    deg_row = nc.dram_tensor("deg_row", [1, nblk * BLK], F32, kind="ExternalInput")
    gidx_d = nc.dram_tensor("gidx", [128, NCHT * 8], I16, kind="ExternalInput")
    ident_d = nc.dram_tensor("ident", [128, 128], BF16, kind="ExternalInput")
    ohs_d = nc.dram_tensor("ohs", [128, NCHT * 128], FP8, kind="ExternalInput")
    y = nc.dram_tensor("y", [NP, OUT], F32, kind="ExternalOutput")

    # layer-2 table rows are padded to 128 cols: dma_gather requires
    # elem_size to be a multiple of 256 bytes
    hs1_stage = nc.dram_tensor("hs1_stage", [NP, HID], BF16)
    hs1_full = nc.dram_tensor("hs1_full", [N, HID], BF16, addr_space="Shared")
    hsr2_stage = nc.dram_tensor("hsr2_stage", [NP, 128], BF16)
    hsr2_full = nc.dram_tensor("hsr2_full", [N, 128], BF16, addr_space="Shared")
    cc_warm_in = nc.dram_tensor("cc_warm_in", [1, 16], BF16)
    cc_warm_out = nc.dram_tensor("cc_warm_out", [ncores, 16], BF16, addr_space="Shared")

    PA, PB = meta["PA"], meta["PB"]
    BA = PA // BLK  # blocks in the first piece

    rg = [list(range(ncores))]
    qn = [0]

    def next_q():
        q = qn[0]
        qn[0] = (q + 1) % NQUEUES
        return q

    with tile.TileContext(nc) as tc:
        with (
            tc.tile_pool(name="const", bufs=1) as constp,
            tc.tile_pool(name="gath", bufs=10) as gathp,
            tc.tile_pool(name="hs", bufs=4) as hsp,
            tc.tile_pool(name="ps", bufs=3, space="PSUM") as psp,
            tc.tile_pool(name="pso", bufs=2, space="PSUM") as psop,
        ):
            # tiny collective first: absorbs the one-time CC barrier /
            # rendezvous (~45us) behind the constant loads and phase B
            warm = constp.tile([1, 16], BF16, tag="warm")
            nc.vector.memset(warm[:], 0.0)
            nc.sync.dma_start(out=cc_warm_in[:, :], in_=warm[:])
            nc.gpsimd.collective_compute(
                "AllGather",
                mybir.AluOpType.bypass,
                replica_groups=rg,
                ins=[cc_warm_in[:, :].opt()],
                outs=[cc_warm_out[:, :].opt()],
            )
            nc.gpsimd.load_library(library_config.mlp)

            # ---- loads needed by phase B ----
            xsb = []
            for k in range(KC):
                t = constp.tile([128, NP], F32, tag=f"x{k}")
                nc.sync.dma_start(out=t[:], in_=x_tr[k * 128 : (k + 1) * 128, :])
                xsb.append(t)
            w1c = []
            for k in range(KC):
                t = constp.tile([128, HID], F32, tag=f"w1c{k}")
                nc.sync.dma_start(out=t[:], in_=w1[k * 128 : (k + 1) * 128, :])
                w1c.append(t)
            dinv_sb = constp.tile([128, nblk], F32, tag="dinv")
            nc.sync.dma_start(out=dinv_sb[:], in_=deg_own[:, :])
            nc.scalar.sqrt(dinv_sb[:], dinv_sb[:])
            nc.vector.reciprocal(dinv_sb[:], dinv_sb[:])

            # ---- phase B: hs1 = dinv * (x @ W1) for own rows ----
            hs1_t = []
            for b in range(nblk):
                w = widths[b]
                ph = psp.tile([128, HID], F32, tag="acc")
                for k in range(KC):
                    nc.tensor.matmul(
                        ph[:w, :],
                        lhsT=xsb[k][:, b * BLK : b * BLK + w],
                        rhs=w1c[k][:, :],
                        start=(k == 0),
                        stop=(k == KC - 1),
                    )
                t = constp.tile([128, HID], BF16, tag=f"hs1_{b}")
                nc.scalar.activation(
                    t[:w, :],
                    ph[:w, :],
                    mybir.ActivationFunctionType.Copy,
                    scale=dinv_sb[:w, b : b + 1],
                )
                nc.sync.dma_start(
                    out=hs1_stage[b * BLK : b * BLK + w, :], in_=t[:w, :]
                )
                hs1_t.append(t)
                if b == BA - 1:
                    # first-piece AllGather overlaps the rest of phase B
                    nc.gpsimd.collective_compute(
                        "AllGather",
                        mybir.AluOpType.bypass,
                        replica_groups=rg,
                        ins=[hs1_stage[0:PA, :].opt()],
                        outs=[hs1_full[0 : ncores * PA, :].opt()],
                    )

            nc.gpsimd.collective_compute(
                "AllGather",
                mybir.AluOpType.bypass,
                replica_groups=rg,
                ins=[hs1_stage[PA:NP, :].opt()],
                outs=[hs1_full[ncores * PA : ncores * NP, :].opt()],
            )

            # ---- remaining constants (overlap the AllGather) ----
            gidx_sb = constp.tile([128, NCHT * 8], I16, tag="gidx")
            nc.sync.dma_start(out=gidx_sb[:], in_=gidx_d[:, :])
            ident_sb = constp.tile([128, 128], BF16, tag="ident")
            nc.sync.dma_start(out=ident_sb[:], in_=ident_d[:, :])

            # one-hot scatter matrices, fp8, SBUF-resident for both layers
            ohs_sb = constp.tile([128, NCHT * 128], FP8, tag="ohs")
            qcols = _cdiv(NCHT * 128, 4)
            for q in range(4):
                c0q = q * qcols
                c1q = min((q + 1) * qcols, NCHT * 128)
                nc.sync.dma_start(out=ohs_sb[:, c0q:c1q], in_=ohs_d[:, c0q:c1q])

            w2_sb = constp.tile([HID, OUT], F32, tag="w2")
            nc.sync.dma_start(out=w2_sb[:], in_=w2[:, :])
            b1_sb = constp.tile([HID, 1], F32, tag="b1")
            nc.sync.dma_start(out=b1_sb[:], in_=b1c[:, :])
            b2_sb = constp.tile([1, OUT], F32, tag="b2")
            nc.sync.dma_start(out=b2_sb[:], in_=b2[:, :])
            ones_sb = constp.tile([1, 128], F32, tag="ones")
            nc.vector.memset(ones_sb[:], 1.0)

            # broadcast b2 to all partitions via rank-1 matmul
            pb2 = psop.tile([128, 128], F32, tag="aux")
            nc.tensor.matmul(pb2[:, :OUT], lhsT=ones_sb[:], rhs=b2_sb[:],
                             start=True, stop=True)
            b2_bc = constp.tile([128, OUT], F32, tag="b2bc")
            nc.vector.tensor_copy(b2_bc[:], pb2[:, :OUT])

            # per-column dinv for the transposed layer-1 epilogue: broadcast
            # deg down partitions per block, then one sqrt + reciprocal
            deg_rsb = constp.tile([1, nblk * BLK], F32, tag="degrow")
            nc.sync.dma_start(out=deg_rsb[:], in_=deg_row[:, :])
            dinv_bc = constp.tile([128, nblk * BLK], F32, tag="dinvbc")
            for b in range(nblk):
                pdv = psop.tile([128, 128], F32, tag="aux")
                nc.tensor.matmul(
                    pdv[:], lhsT=ones_sb[:],
                    rhs=deg_rsb[:, b * BLK : (b + 1) * BLK],
                    start=True, stop=True,
                )
                nc.vector.tensor_copy(dinv_bc[:, b * BLK : (b + 1) * BLK], pdv[:])
            nc.scalar.sqrt(dinv_bc[:], dinv_bc[:])
            nc.vector.reciprocal(dinv_bc[:], dinv_bc[:])

            # ---- phase D: S1^T -> hsr^T -> hsr2 = dinv*(hsr @ W2) ----
            segi = [0]
            hsr2_t = []
            for b in range(nblk):
                w = widths[b]
                ch = CH[b]
                c0 = cofs[b]
                p1 = psp.tile([128, 128], F32, tag="acc")
                # self-loop: S1^T += hs1[own block]^T via identity matmul
                nc.tensor.matmul(
                    p1[:, :w], lhsT=hs1_t[b][:w, :], rhs=ident_sb[:w, :w],
                    start=True, stop=False,
                )
                for s0 in range(0, ch, MAXCH):
                    sch = min(MAXCH, ch - s0)
                    nv = seg_valid[segi[0]]
                    segi[0] += 1
                    g1 = gathp.tile([128, MAXCH, HID], BF16, tag="g")
                    nc.gpsimd.dma_gather(
                        g1[:, :sch, :],
                        hs1_full.ap(),
                        gidx_sb[:, (c0 + s0) * 8 : (c0 + s0 + sch) * 8],
                        sch * 128,
                        nv,
                        HID,
                        queue_num=next_q(),
                    )
                    for c in range(sch):
                        vk = min(128, nv - c * 128)
                        nc.tensor.matmul(
                            p1[:, :w],
                            lhsT=g1[:vk, c, :],
                            rhs=ohs_sb[:vk, (c0 + s0 + c) * 128 : (c0 + s0 + c) * 128 + w],
                            start=False,
                            stop=(s0 + c == ch - 1),
                        )
                # hsr^T = relu(dinv_col * S1^T + b1); hsr2 = dinv*(hsr @ W2)
                t1 = hsp.tile([128, 128], F32, tag="t1")
                nc.vector.tensor_tensor(
                    out=t1[:, :w], in0=p1[:, :w],
                    in1=dinv_bc[:, b * BLK : b * BLK + w],
                    op=mybir.AluOpType.mult,
                )
                hsrT = hsp.tile([128, 128], F32, tag="hsrT")
                nc.scalar.activation(
                    hsrT[:, :w], t1[:, :w],
                    mybir.ActivationFunctionType.Relu,
                    bias=b1_sb[:, 0:1],
                )
                p2s = psop.tile([128, 128], F32, tag="aux")
                nc.tensor.matmul(
                    p2s[:w, :OUT], lhsT=hsrT[:, :w], rhs=w2_sb[:, :],
                    start=True, stop=True,
                )
                t2 = constp.tile([128, 128], BF16, tag=f"hsr2_{b}")
                nc.vector.memset(t2[:, OUT:], 0.0)
                nc.scalar.activation(
                    t2[:w, :OUT], p2s[:w, :OUT],
                    mybir.ActivationFunctionType.Copy,
                    scale=dinv_sb[:w, b : b + 1],
                )
                nc.sync.dma_start(
                    out=hsr2_stage[b * BLK : b * BLK + w, :], in_=t2[:w, :]
                )
                hsr2_t.append(t2)
                if b == BA - 1:
                    # first-piece AllGather overlaps the rest of phase D
                    nc.gpsimd.collective_compute(
                        "AllGather",
                        mybir.AluOpType.bypass,
                        replica_groups=rg,
                        ins=[hsr2_stage[0:PA, :].opt()],
                        outs=[hsr2_full[0 : ncores * PA, :].opt()],
                    )

            nc.gpsimd.collective_compute(
                "AllGather",
                mybir.AluOpType.bypass,
                replica_groups=rg,
                ins=[hsr2_stage[PA:NP, :].opt()],
                outs=[hsr2_full[ncores * PA : ncores * NP, :].opt()],
            )

            # ---- phase F: S2 -> y ----
            segi[0] = 0
            for b in range(nblk):
                w = widths[b]
                ch = CH[b]
                c0 = cofs[b]
                p2 = psp.tile([128, 128], F32, tag="acc")
                # self-loop: S2 += hsr2[own block] via identity matmul
                nc.tensor.matmul(
                    p2[:w, :OUT], lhsT=ident_sb[:w, :w], rhs=hsr2_t[b][:w, :OUT],
                    start=True, stop=False,
                )
                for s0 in range(0, ch, MAXCH):
                    sch = min(MAXCH, ch - s0)
                    nv = seg_valid[segi[0]]
                    segi[0] += 1
                    g2 = gathp.tile([128, MAXCH, 128], BF16, tag="g")
                    nc.gpsimd.dma_gather(
                        g2[:, :sch, :],
                        hsr2_full.ap(),
                        gidx_sb[:, (c0 + s0) * 8 : (c0 + s0 + sch) * 8],
                        sch * 128,
                        nv,
                        128,
                        queue_num=next_q(),
                    )
                    for c in range(sch):
                        vk = min(128, nv - c * 128)
                        nc.tensor.matmul(
                            p2[:w, :OUT],
                            lhsT=ohs_sb[:vk, (c0 + s0 + c) * 128 : (c0 + s0 + c) * 128 + w],
                            rhs=g2[:vk, c, :OUT],
                            start=False,
                            stop=(s0 + c == ch - 1),
                        )
                # y = dinv*S2 + b2
                o1 = hsp.tile([128, OUT], F32, tag="o1")
                nc.scalar.activation(
                    o1[:w, :], p2[:w, :OUT],
                    mybir.ActivationFunctionType.Copy,
                    scale=dinv_sb[:w, b : b + 1],
                )
                yt = hsp.tile([128, OUT], F32, tag="yt")
                nc.vector.tensor_tensor(
                    out=yt[:w, :], in0=o1[:w, :], in1=b2_bc[:w, :],
                    op=mybir.AluOpType.add,
                )
                nc.sync.dma_start(out=y[b * BLK : b * BLK + w, :], in_=yt[:w, :])

    nc.compile()
    return nc


def _make_ident():
    import ml_dtypes

    return np.eye(128, dtype=np.float32).astype(ml_dtypes.bfloat16)


_IDENT = _make_ident()


def make_in_maps(per_core, W1, b1, W2, b2):
    W1 = np.ascontiguousarray(np.asarray(W1, np.float32))
    W2 = np.ascontiguousarray(np.asarray(W2, np.float32))
    b1 = np.ascontiguousarray(np.asarray(b1, np.float32).reshape(-1, 1))
    b2 = np.asarray(b2, np.float32).reshape(1, -1)
    return [
        {
            "x_tr": pc["x_tr"],
            "w1": W1,
            "b1c": b1,
            "w2": W2,
            "b2": b2,
            "deg_own": pc["deg_own"],
            "deg_row": pc["deg_row"],
            "gidx": pc["gidx"],
            "ohs": pc["ohs"],
            "ident": _IDENT,
        }
        for pc in per_core
    ]


def kernel_run(x, edge_index, W1, b1, W2, b2, trace=False):
    x = np.ascontiguousarray(np.asarray(x, np.float32))
    per_core, meta = preprocess(x, edge_index)
    HID = np.asarray(W1).shape[1]
    OUT = np.asarray(W2).shape[1]
    nc = build_nc(meta, HID, OUT)
    in_maps = make_in_maps(per_core, W1, b1, W2, b2)
    res = run_bass_kernel_spmd(nc, in_maps, core_ids=list(range(NCORES)), trace=trace)
    out = np.concatenate([r["y"] for r in res.results], axis=0)
    return out, res


def kernel(x, edge_index, W1, b1, W2, b2):
    out, _ = kernel_run(x, edge_index, W1, b1, W2, b2)
    return out


# revision 14
# speedup vs baseline: 1.2505x; 1.2505x over previous
"""2-layer GCN encoder as a distributed Bass kernel on 8 TRN2 NeuronCores.

Decomposition (per core, nodes sharded by destination):
  hs1[v]  = dinv[v] * (x[v] @ W1)                 (own rows, AllGather, bf16)
  S1T[:,d]= sum_{e: dst=d} hs1[src_e]             (dma_gather + one-hot matmul,
                                                   accumulated TRANSPOSED)
  hsrT    = relu(dinv_col * S1T + b1)             (dst-side dinv per column,
                                                   bias per partition)
  hsr2[v] = dinv[v] * (hsrT^T @ W2)               (W2 commutes with the layer-2
                                                   edge sum -> aggregate at 64)
  S2[d]   = sum_{e: dst=d} hsr2[src_e]            (AllGather + gather at 64 wide)
  y[d]    = dinv[d]*S2[d] + b2

The one-hot scatter matrices (0/1/multiplicity) are streamed once as fp8_e4m3
and kept SBUF-resident for both layers (mixed fp8 x bf16 matmul is exact for
these values).  All float normalization (dinv = rsqrt(deg)) is computed on
device.  Edge gathers use SWDGE dma_gather (<=1024 indices per instruction,
round-robin over 4 SWDGE queues).  Self-loops are extra identity matmuls on
the block's own SBUF-kept tiles.
"""

import numpy as np

import concourse.bass as bass
import concourse.bacc as bacc
import concourse.mybir as mybir
import concourse.tile as tile
from concourse import library_config
from concourse.bass_utils import run_bass_kernel_spmd

F32 = mybir.dt.float32
BF16 = mybir.dt.bfloat16
FP8 = mybir.dt.float8e4
I16 = mybir.dt.int16

NCORES = 8
BLK = 128
# Max 128-index chunks per dma_gather instruction: the SWDGE descriptor
# ring holds only ~100 descriptors per DMA engine and a gather generates
# num_idxs/16 per ring; >=1792 indices hangs the ring-reclaim wait in the
# Q7 decode and wedges the device.  1024 is safe.  The ucode keeps ONE
# gather in flight per queue regardless of ring space, so smaller
# instructions only add per-cycle overhead.
MAXCH = 8
NQUEUES = 4


def _cdiv(a, b):
    return (a + b - 1) // b


def preprocess(x, edge_index, ncores=NCORES):
    """Host-side graph partitioning: shard nodes/edges by dst, sort edges,
    build per-core gather indices (SWDGE wrapped layout), the 0/1 one-hot
    chunk matrices (fp8), and per-node degree counts.  Index/structure work
    plus dtype casts only; all float math happens on device."""
    import ml_dtypes

    N, IN = x.shape
    assert N % ncores == 0
    NP = N // ncores
    nblk = _cdiv(NP, BLK)

    src = np.asarray(edge_index[0], dtype=np.int64)
    dst = np.asarray(edge_index[1], dtype=np.int64)
    # degree includes the self-loop, but self-loops are handled by an
    # identity matmul on device, not by the edge gather
    deg = (np.bincount(dst, minlength=N) + 1).astype(np.float32)

    # dedupe repeated (src, dst) pairs; multiplicity goes into the multi-hot
    key = dst * N + src
    ukey, mult = np.unique(key, return_counts=True)
    dst_s = ukey // N
    src_s = ukey % N

    bounds = np.array(
        [i * NP + b * BLK for i in range(ncores) for b in range(nblk)] + [N],
        dtype=np.int64,
    )
    pos = np.searchsorted(dst_s, bounds)

    # dedupe sources within each (core, block): the one-hot row scatters to
    # all of that source's dst columns, and gathering each unique src once
    # in ascending order improves DRAM locality.  The one-hot absorbs any
    # edge permutation.
    blk_usrc = {}
    blk_scatter = {}  # (p, col_within_block, value) triplets
    ucnt = np.zeros((ncores, nblk), np.int64)
    for i in range(ncores):
        for b in range(nblk):
            k = i * nblk + b
            s0, s1 = pos[k], pos[k + 1]
            usrc, inv = np.unique(src_s[s0:s1], return_inverse=True)
            ucnt[i, b] = len(usrc)
            dl = (dst_s[s0:s1] - (i * NP + b * BLK)).astype(np.int64)
            blk_usrc[i, b] = usrc
            blk_scatter[i, b] = (inv, dl, mult[s0:s1].astype(np.float32))

    # uniform chunk counts across cores (SPMD: one program for all cores)
    CH = np.maximum(1, _cdiv(ucnt.max(axis=0), 128)).astype(np.int64)
    cofs = np.concatenate([[0], np.cumsum(CH)]).astype(np.int64)
    NCHT = int(CH.sum())
    widths = [min(BLK, NP - b * BLK) for b in range(nblk)]

    # per-segment valid counts must be uniform across cores: num_idxs_reg is
    # baked into the shared program, and the ucode asserts it equals the
    # count of non-negative indices
    segs = []  # (block, seg_start_chunk, seg_chunks)
    for b in range(nblk):
        for s0 in range(0, int(CH[b]), MAXCH):
            segs.append((b, s0, min(MAXCH, int(CH[b]) - s0)))
    seg_valid = []
    for (b, s0, sch) in segs:
        lo = s0 * 128
        v = max(min(int(ucnt[i, b]) - lo, sch * 128) for i in range(ncores))
        seg_valid.append(max(v, 1))

    # The SWDGE ucode assigns logical index position i to descriptor ring
    # i % 16 (one ring per SDMA engine).  Permute each segment's indices so
    # ring r gets a CONTIGUOUS ascending slice of the sorted sources: each
    # engine's descriptor stream then walks HBM monotonically instead of
    # jumping by 16 rows per descriptor.
    def ring_perm(nv):
        # logical position for sorted slot q in [0, nv)
        n_r = (nv - np.arange(16) + 15) // 16  # slots per ring
        off = np.concatenate([[0], np.cumsum(n_r)])
        i_of_q = np.empty(nv, np.int64)
        for r in range(16):
            j = np.arange(n_r[r])
            i_of_q[off[r] + j] = j * 16 + r
        return i_of_q

    seg_perm = {sv: ring_perm(sv) for sv in set(seg_valid)}

    per_core = []
    for i in range(ncores):
        gidx = np.full((128, NCHT * 8), -1, np.int16)
        ohs = np.zeros((128, NCHT * 128), np.float32)
        for b in range(nblk):
            usrc = blk_usrc[i, b]
            inv, dl, mlt = blk_scatter[i, b]
            ne = len(usrc)
            # logical position of each sorted slot within its block,
            # segment by segment
            slot_to_i = np.empty(int(CH[b]) * 128, np.int64)
            for si, (bb, s0, sch) in enumerate(segs):
                if bb != b:
                    continue
                lo = s0 * 128
                nv = seg_valid[si]
                perm = seg_perm[nv]
                slot_to_i[lo : lo + nv] = lo + perm
                slot_to_i[lo + nv : lo + sch * 128] = np.arange(lo + nv, lo + sch * 128)
                # this core's values for the segment: real sources then
                # index-0 padding up to nv, then -1 up to capacity
                nreal = min(max(ne - lo, 0), sch * 128)
                vals = np.full(sch * 128, -1, np.int64)
                vals[perm[:nreal]] = usrc[lo : lo + nreal]
                vals[perm[nreal:nv]] = 0
                wr = vals.reshape(sch * 8, 16).T  # [16, sch*8]
                gidx[:, (cofs[b] + s0) * 8 : (cofs[b] + s0 + sch) * 8] = np.tile(
                    wr, (8, 1)
                )
            # multi-hot: the edge whose unique src sits at logical position
            # i adds its multiplicity at (partition i%128, chunk i//128,
            # col = local dst)
            iq = slot_to_i[inv]
            c = iq // 128
            p = iq % 128
            np.add.at(ohs, (p, (cofs[b] + c) * 128 + dl), mlt)
        degp = np.concatenate(
            [deg[i * NP : (i + 1) * NP], np.ones(nblk * BLK - NP, np.float32)]
        )
        per_core.append(
            {
                "x_tr": np.ascontiguousarray(x[i * NP : (i + 1) * NP].T),
                "deg_own": np.ascontiguousarray(degp.reshape(nblk, BLK).T),
                "deg_row": np.ascontiguousarray(degp.reshape(1, nblk * BLK)),
                "gidx": gidx,
                "ohs": ohs.astype(ml_dtypes.float8_e4m3),
            }
        )

    meta = {
        "seg_valid": seg_valid,
        "N": N,
        "NP": NP,
        "IN": IN,
        "nblk": nblk,
        "CH": [int(c) for c in CH],
        "cofs": [int(c) for c in cofs],
        "widths": widths,
        "NCHT": NCHT,
    }
    return per_core, meta


def build_nc(meta, HID, OUT, ncores=NCORES):
    N, NP, IN = meta["N"], meta["NP"], meta["IN"]
    nblk, CH, cofs, widths = meta["nblk"], meta["CH"], meta["cofs"], meta["widths"]
    seg_valid = meta["seg_valid"]
    NCHT = meta["NCHT"]
    KC = IN // 128
    assert IN % 128 == 0 and HID == 128 and OUT <= 512

    nc = bacc.Bacc(
        "TRN2",
        target_bir_lowering=False,
        debug=False,
        num_devices=ncores,
        num_swdge_queues=NQUEUES,
    )

    x_tr = nc.dram_tensor("x_tr", [IN, NP], F32, kind="ExternalInput")
    w1 = nc.dram_tensor("w1", [IN, HID], F32, kind="ExternalInput")
    b1c = nc.dram_tensor("b1c", [HID, 1], F32, kind="ExternalInput")
    w2 = nc.dram_tensor("w2", [HID, OUT], F32, kind="ExternalInput")
    b2 = nc.dram_tensor("b2", [1, OUT], F32, kind="ExternalInput")
    deg_own = nc.dram_tensor("deg_own", [128, nblk], F32, kind="ExternalInput")
    deg_row = nc.dram_tensor("deg_row", [1, nblk * BLK], F32, kind="ExternalInput")
    gidx_d = nc.dram_tensor("gidx", [128, NCHT * 8], I16, kind="ExternalInput")
    ident_d = nc.dram_tensor("ident", [128, 128], BF16, kind="ExternalInput")
    ohs_d = nc.dram_tensor("ohs", [128, NCHT * 128], FP8, kind="ExternalInput")
    y = nc.dram_tensor("y", [NP, OUT], F32, kind="ExternalOutput")

    # layer-2 table rows are padded to 128 cols: dma_gather requires
    # elem_size to be a multiple of 256 bytes
    hs1_stage = nc.dram_tensor("hs1_stage", [NP, HID], BF16)
    hs1_full = nc.dram_tensor("hs1_full", [N, HID], BF16, addr_space="Shared")
    hsr2_stage = nc.dram_tensor("hsr2_stage", [NP, 128], BF16)
    hsr2_full = nc.dram_tensor("hsr2_full", [N, 128], BF16, addr_space="Shared")
    rg = [list(range(ncores))]
    qn = [0]

    def next_q():
        q = qn[0]
        qn[0] = (q + 1) % NQUEUES
        return q

    with tile.TileContext(nc) as tc:
        with (
            tc.tile_pool(name="const", bufs=1) as constp,
            tc.tile_pool(name="gath", bufs=10) as gathp,
            tc.tile_pool(name="hs", bufs=4) as hsp,
            tc.tile_pool(name="ps", bufs=3, space="PSUM") as psp,
            tc.tile_pool(name="pso", bufs=2, space="PSUM") as psop,
        ):
            nc.gpsimd.load_library(library_config.mlp)

            # ---- loads needed by phase B ----
            xsb = []
            for k in range(KC):
                t = constp.tile([128, NP], F32, tag=f"x{k}")
                nc.sync.dma_start(out=t[:], in_=x_tr[k * 128 : (k + 1) * 128, :])
                xsb.append(t)
            w1c = []
            for k in range(KC):
                t = constp.tile([128, HID], F32, tag=f"w1c{k}")
                nc.sync.dma_start(out=t[:], in_=w1[k * 128 : (k + 1) * 128, :])
                w1c.append(t)
            dinv_sb = constp.tile([128, nblk], F32, tag="dinv")
            nc.sync.dma_start(out=dinv_sb[:], in_=deg_own[:, :])
            nc.scalar.sqrt(dinv_sb[:], dinv_sb[:])
            nc.vector.reciprocal(dinv_sb[:], dinv_sb[:])

            # ---- phase B: hs1 = dinv * (x @ W1) for own rows ----
            hs1_t = []
            for b in range(nblk):
                w = widths[b]
                ph = psp.tile([128, HID], F32, tag="acc")
                for k in range(KC):
                    nc.tensor.matmul(
                        ph[:w, :],
                        lhsT=xsb[k][:, b * BLK : b * BLK + w],
                        rhs=w1c[k][:, :],
                        start=(k == 0),
                        stop=(k == KC - 1),
                    )
                t = constp.tile([128, HID], BF16, tag=f"hs1_{b}")
                nc.scalar.activation(
                    t[:w, :],
                    ph[:w, :],
                    mybir.ActivationFunctionType.Copy,
                    scale=dinv_sb[:w, b : b + 1],
                )
                nc.sync.dma_start(
                    out=hs1_stage[b * BLK : b * BLK + w, :], in_=t[:w, :]
                )
                hs1_t.append(t)

            nc.gpsimd.collective_compute(
                "AllGather",
                mybir.AluOpType.bypass,
                replica_groups=rg,
                ins=[hs1_stage[0:NP, :].opt()],
                outs=[hs1_full[0 : ncores * NP, :].opt()],
            )

            # ---- remaining constants (overlap the AllGather) ----
            gidx_sb = constp.tile([128, NCHT * 8], I16, tag="gidx")
            nc.sync.dma_start(out=gidx_sb[:], in_=gidx_d[:, :])
            ident_sb = constp.tile([128, 128], BF16, tag="ident")
            nc.sync.dma_start(out=ident_sb[:], in_=ident_d[:, :])

            # one-hot scatter matrices, fp8, SBUF-resident for both layers
            ohs_sb = constp.tile([128, NCHT * 128], FP8, tag="ohs")
            qcols = _cdiv(NCHT * 128, 4)
            for q in range(4):
                c0q = q * qcols
                c1q = min((q + 1) * qcols, NCHT * 128)
                nc.sync.dma_start(out=ohs_sb[:, c0q:c1q], in_=ohs_d[:, c0q:c1q])

            w2_sb = constp.tile([HID, OUT], F32, tag="w2")
            nc.sync.dma_start(out=w2_sb[:], in_=w2[:, :])
            b1_sb = constp.tile([HID, 1], F32, tag="b1")
            nc.sync.dma_start(out=b1_sb[:], in_=b1c[:, :])
            b2_sb = constp.tile([1, OUT], F32, tag="b2")
            nc.sync.dma_start(out=b2_sb[:], in_=b2[:, :])
            ones_sb = constp.tile([1, 128], F32, tag="ones")
            nc.vector.memset(ones_sb[:], 1.0)

            # broadcast b2 to all partitions via rank-1 matmul
            pb2 = psop.tile([128, 128], F32, tag="aux")
            nc.tensor.matmul(pb2[:, :OUT], lhsT=ones_sb[:], rhs=b2_sb[:],
                             start=True, stop=True)
            b2_bc = constp.tile([128, OUT], F32, tag="b2bc")
            nc.vector.tensor_copy(b2_bc[:], pb2[:, :OUT])

            # per-column dinv for the transposed layer-1 epilogue: broadcast
            # deg down partitions per block, then one sqrt + reciprocal
            deg_rsb = constp.tile([1, nblk * BLK], F32, tag="degrow")
            nc.sync.dma_start(out=deg_rsb[:], in_=deg_row[:, :])
            dinv_bc = constp.tile([128, nblk * BLK], F32, tag="dinvbc")
            for b in range(nblk):
                pdv = psop.tile([128, 128], F32, tag="aux")
                nc.tensor.matmul(
                    pdv[:], lhsT=ones_sb[:],
                    rhs=deg_rsb[:, b * BLK : (b + 1) * BLK],
                    start=True, stop=True,
                )
                nc.vector.tensor_copy(dinv_bc[:, b * BLK : (b + 1) * BLK], pdv[:])
            nc.scalar.sqrt(dinv_bc[:], dinv_bc[:])
            nc.vector.reciprocal(dinv_bc[:], dinv_bc[:])

            # ---- phase D: S1^T -> hsr^T -> hsr2 = dinv*(hsr @ W2) ----
            segi = [0]
            hsr2_t = []
            for b in range(nblk):
                w = widths[b]
                ch = CH[b]
                c0 = cofs[b]
                p1 = psp.tile([128, 128], F32, tag="acc")
                # self-loop: S1^T += hs1[own block]^T via identity matmul
                nc.tensor.matmul(
                    p1[:, :w], lhsT=hs1_t[b][:w, :], rhs=ident_sb[:w, :w],
                    start=True, stop=False,
                )
                for s0 in range(0, ch, MAXCH):
                    sch = min(MAXCH, ch - s0)
                    nv = seg_valid[segi[0]]
                    segi[0] += 1
                    g1 = gathp.tile([128, MAXCH, HID], BF16, tag="g")
                    nc.gpsimd.dma_gather(
                        g1[:, :sch, :],
                        hs1_full.ap(),
                        gidx_sb[:, (c0 + s0) * 8 : (c0 + s0 + sch) * 8],
                        sch * 128,
                        nv,
                        HID,
                        single_packet=False,
                        queue_num=next_q(),
                    )
                    for c in range(sch):
                        vk = min(128, nv - c * 128)
                        nc.tensor.matmul(
                            p1[:, :w],
                            lhsT=g1[:vk, c, :],
                            rhs=ohs_sb[:vk, (c0 + s0 + c) * 128 : (c0 + s0 + c) * 128 + w],
                            start=False,
                            stop=(s0 + c == ch - 1),
                        )
                # hsr^T = relu(dinv_col * S1^T + b1); hsr2 = dinv*(hsr @ W2)
                t1 = hsp.tile([128, 128], F32, tag="t1")
                nc.vector.tensor_tensor(
                    out=t1[:, :w], in0=p1[:, :w],
                    in1=dinv_bc[:, b * BLK : b * BLK + w],
                    op=mybir.AluOpType.mult,
                )
                hsrT = hsp.tile([128, 128], F32, tag="hsrT")
                nc.scalar.activation(
                    hsrT[:, :w], t1[:, :w],
                    mybir.ActivationFunctionType.Relu,
                    bias=b1_sb[:, 0:1],
                )
                p2s = psop.tile([128, 128], F32, tag="aux")
                nc.tensor.matmul(
                    p2s[:w, :OUT], lhsT=hsrT[:, :w], rhs=w2_sb[:, :],
                    start=True, stop=True,
                )
                t2 = constp.tile([128, 128], BF16, tag=f"hsr2_{b}")
                nc.vector.memset(t2[:, OUT:], 0.0)
                nc.scalar.activation(
                    t2[:w, :OUT], p2s[:w, :OUT],
                    mybir.ActivationFunctionType.Copy,
                    scale=dinv_sb[:w, b : b + 1],
                )
                nc.sync.dma_start(
                    out=hsr2_stage[b * BLK : b * BLK + w, :], in_=t2[:w, :]
                )
                hsr2_t.append(t2)

            nc.gpsimd.collective_compute(
                "AllGather",
                mybir.AluOpType.bypass,
                replica_groups=rg,
                ins=[hsr2_stage[0:NP, :].opt()],
                outs=[hsr2_full[0 : ncores * NP, :].opt()],
            )

            # ---- phase F: S2 -> y ----
            segi[0] = 0
            for b in range(nblk):
                w = widths[b]
                ch = CH[b]
                c0 = cofs[b]
                p2 = psp.tile([128, 128], F32, tag="acc")
                # self-loop: S2 += hsr2[own block] via identity matmul
                nc.tensor.matmul(
                    p2[:w, :OUT], lhsT=ident_sb[:w, :w], rhs=hsr2_t[b][:w, :OUT],
                    start=True, stop=False,
                )
                for s0 in range(0, ch, MAXCH):
                    sch = min(MAXCH, ch - s0)
                    nv = seg_valid[segi[0]]
                    segi[0] += 1
                    g2 = gathp.tile([128, MAXCH, 128], BF16, tag="g")
                    nc.gpsimd.dma_gather(
                        g2[:, :sch, :],
                        hsr2_full.ap(),
                        gidx_sb[:, (c0 + s0) * 8 : (c0 + s0 + sch) * 8],
                        sch * 128,
                        nv,
                        128,
                        single_packet=False,
                        queue_num=next_q(),
                    )
                    for c in range(sch):
                        vk = min(128, nv - c * 128)
                        nc.tensor.matmul(
                            p2[:w, :OUT],
                            lhsT=ohs_sb[:vk, (c0 + s0 + c) * 128 : (c0 + s0 + c) * 128 + w],
                            rhs=g2[:vk, c, :OUT],
                            start=False,
                            stop=(s0 + c == ch - 1),
                        )
                # y = dinv*S2 + b2
                o1 = hsp.tile([128, OUT], F32, tag="o1")
                nc.scalar.activation(
                    o1[:w, :], p2[:w, :OUT],
                    mybir.ActivationFunctionType.Copy,
                    scale=dinv_sb[:w, b : b + 1],
                )
                yt = hsp.tile([128, OUT], F32, tag="yt")
                nc.vector.tensor_tensor(
                    out=yt[:w, :], in0=o1[:w, :], in1=b2_bc[:w, :],
                    op=mybir.AluOpType.add,
                )
                nc.sync.dma_start(out=y[b * BLK : b * BLK + w, :], in_=yt[:w, :])

    nc.compile()
    return nc


def _make_ident():
    import ml_dtypes

    return np.eye(128, dtype=np.float32).astype(ml_dtypes.bfloat16)


_IDENT = _make_ident()


def make_in_maps(per_core, W1, b1, W2, b2):
    W1 = np.ascontiguousarray(np.asarray(W1, np.float32))
    W2 = np.ascontiguousarray(np.asarray(W2, np.float32))
    b1 = np.ascontiguousarray(np.asarray(b1, np.float32).reshape(-1, 1))
    b2 = np.asarray(b2, np.float32).reshape(1, -1)
    return [
        {
            "x_tr": pc["x_tr"],
            "w1": W1,
            "b1c": b1,
            "w2": W2,
            "b2": b2,
            "deg_own": pc["deg_own"],
            "deg_row": pc["deg_row"],
            "gidx": pc["gidx"],
            "ohs": pc["ohs"],
            "ident": _IDENT,
        }
        for pc in per_core
    ]


def kernel_run(x, edge_index, W1, b1, W2, b2, trace=False):
    x = np.ascontiguousarray(np.asarray(x, np.float32))
    per_core, meta = preprocess(x, edge_index)
    HID = np.asarray(W1).shape[1]
    OUT = np.asarray(W2).shape[1]
    nc = build_nc(meta, HID, OUT)
    in_maps = make_in_maps(per_core, W1, b1, W2, b2)
    res = run_bass_kernel_spmd(nc, in_maps, core_ids=list(range(NCORES)), trace=trace)
    out = np.concatenate([r["y"] for r in res.results], axis=0)
    return out, res


def kernel(x, edge_index, W1, b1, W2, b2):
    out, _ = kernel_run(x, edge_index, W1, b1, W2, b2)
    return out


# revision 15
# speedup vs baseline: 1.3544x; 1.0831x over previous
"""2-layer GCN encoder as a distributed Bass kernel on 8 TRN2 NeuronCores.

Decomposition (per core, nodes sharded by destination):
  hs1[v]  = dinv[v] * (x[v] @ W1)                 (own rows, AllGather, bf16)
  S1T[:,d]= sum_{e: dst=d} hs1[src_e]             (dma_gather + one-hot matmul,
                                                   accumulated TRANSPOSED)
  hsrT    = relu(dinv_col * S1T + b1)             (dst-side dinv per column,
                                                   bias per partition)
  hsr2[v] = dinv[v] * (hsrT^T @ W2)               (W2 commutes with the layer-2
                                                   edge sum -> aggregate at 64)
  S2[d]   = sum_{e: dst=d} hsr2[src_e]            (AllGather + gather at 64 wide)
  y[d]    = dinv[d]*S2[d] + b2

The one-hot scatter matrices (0/1/multiplicity) are streamed once as fp8_e4m3
and kept SBUF-resident for both layers (mixed fp8 x bf16 matmul is exact for
these values).  All float normalization (dinv = rsqrt(deg)) is computed on
device.  Edge gathers use SWDGE dma_gather (<=1024 indices per instruction,
round-robin over 4 SWDGE queues).  Self-loops are extra identity matmuls on
the block's own SBUF-kept tiles.
"""

import numpy as np

import concourse.bass as bass
import concourse.bacc as bacc
import concourse.mybir as mybir
import concourse.tile as tile
from concourse import library_config
from concourse.bass_utils import run_bass_kernel_spmd

F32 = mybir.dt.float32
BF16 = mybir.dt.bfloat16
FP8 = mybir.dt.float8e4
I16 = mybir.dt.int16

NCORES = 8
BLK = 128
# Max 128-index chunks per dma_gather instruction: the SWDGE descriptor
# ring holds only ~100 descriptors per DMA engine and a gather generates
# num_idxs/16 per ring; >=1792 indices hangs the ring-reclaim wait in the
# Q7 decode and wedges the device.  1024 is safe.  The ucode keeps ONE
# gather in flight per queue regardless of ring space, so smaller
# instructions only add per-cycle overhead.
MAXCH = 8
NQUEUES = 4


def _cdiv(a, b):
    return (a + b - 1) // b


def preprocess(x, edge_index, ncores=NCORES):
    """Host-side graph partitioning: shard nodes/edges by dst, sort edges,
    build per-core gather indices (SWDGE wrapped layout), the 0/1 one-hot
    chunk matrices (fp8), and per-node degree counts.  Index/structure work
    plus dtype casts only; all float math happens on device."""
    import ml_dtypes

    N, IN = x.shape
    assert N % ncores == 0
    NP = N // ncores
    nblk = _cdiv(NP, BLK)

    src = np.asarray(edge_index[0], dtype=np.int64)
    dst = np.asarray(edge_index[1], dtype=np.int64)
    # degree includes the self-loop, but self-loops are handled by an
    # identity matmul on device, not by the edge gather
    deg = (np.bincount(dst, minlength=N) + 1).astype(np.float32)

    # dedupe repeated (src, dst) pairs; multiplicity goes into the multi-hot
    key = dst * N + src
    ukey, mult = np.unique(key, return_counts=True)
    dst_s = ukey // N
    src_s = ukey % N

    bounds = np.array(
        [i * NP + b * BLK for i in range(ncores) for b in range(nblk)] + [N],
        dtype=np.int64,
    )
    pos = np.searchsorted(dst_s, bounds)

    # dedupe sources within each (core, block): the one-hot row scatters to
    # all of that source's dst columns, and gathering each unique src once
    # in ascending order improves DRAM locality.  The one-hot absorbs any
    # edge permutation.
    blk_usrc = {}
    blk_scatter = {}  # (p, col_within_block, value) triplets
    ucnt = np.zeros((ncores, nblk), np.int64)
    for i in range(ncores):
        for b in range(nblk):
            k = i * nblk + b
            s0, s1 = pos[k], pos[k + 1]
            usrc, inv = np.unique(src_s[s0:s1], return_inverse=True)
            ucnt[i, b] = len(usrc)
            dl = (dst_s[s0:s1] - (i * NP + b * BLK)).astype(np.int64)
            blk_usrc[i, b] = usrc
            blk_scatter[i, b] = (inv, dl, mult[s0:s1].astype(np.float32))

    # uniform chunk counts across cores (SPMD: one program for all cores)
    CH = np.maximum(1, _cdiv(ucnt.max(axis=0), 128)).astype(np.int64)
    cofs = np.concatenate([[0], np.cumsum(CH)]).astype(np.int64)
    NCHT = int(CH.sum())
    widths = [min(BLK, NP - b * BLK) for b in range(nblk)]

    # per-segment valid counts must be uniform across cores: num_idxs_reg is
    # baked into the shared program, and the ucode asserts it equals the
    # count of non-negative indices
    segs = []  # (block, seg_start_chunk, seg_chunks)
    for b in range(nblk):
        for s0 in range(0, int(CH[b]), MAXCH):
            segs.append((b, s0, min(MAXCH, int(CH[b]) - s0)))
    seg_valid = []
    for (b, s0, sch) in segs:
        lo = s0 * 128
        v = max(min(int(ucnt[i, b]) - lo, sch * 128) for i in range(ncores))
        seg_valid.append(max(v, 1))

    # The SWDGE ucode assigns logical index position i to descriptor ring
    # i % 16 (one ring per SDMA engine).  Permute each segment's indices so
    # ring r gets a CONTIGUOUS ascending slice of the sorted sources: each
    # engine's descriptor stream then walks HBM monotonically instead of
    # jumping by 16 rows per descriptor.
    def ring_perm(nv):
        # logical position for sorted slot q in [0, nv)
        n_r = (nv - np.arange(16) + 15) // 16  # slots per ring
        off = np.concatenate([[0], np.cumsum(n_r)])
        i_of_q = np.empty(nv, np.int64)
        for r in range(16):
            j = np.arange(n_r[r])
            i_of_q[off[r] + j] = j * 16 + r
        return i_of_q

    seg_perm = {sv: ring_perm(sv) for sv in set(seg_valid)}

    per_core = []
    for i in range(ncores):
        gidx = np.full((128, NCHT * 8), -1, np.int16)
        ohs = np.zeros((128, NCHT * 128), np.float32)
        for b in range(nblk):
            usrc = blk_usrc[i, b]
            inv, dl, mlt = blk_scatter[i, b]
            ne = len(usrc)
            # logical position of each sorted slot within its block,
            # segment by segment
            slot_to_i = np.empty(int(CH[b]) * 128, np.int64)
            for si, (bb, s0, sch) in enumerate(segs):
                if bb != b:
                    continue
                lo = s0 * 128
                nv = seg_valid[si]
                perm = seg_perm[nv]
                slot_to_i[lo : lo + nv] = lo + perm
                slot_to_i[lo + nv : lo + sch * 128] = np.arange(lo + nv, lo + sch * 128)
                # this core's values for the segment: real sources then
                # index-0 padding up to nv, then -1 up to capacity
                nreal = min(max(ne - lo, 0), sch * 128)
                vals = np.full(sch * 128, -1, np.int64)
                vals[perm[:nreal]] = usrc[lo : lo + nreal]
                vals[perm[nreal:nv]] = 0
                wr = vals.reshape(sch * 8, 16).T  # [16, sch*8]
                gidx[:, (cofs[b] + s0) * 8 : (cofs[b] + s0 + sch) * 8] = np.tile(
                    wr, (8, 1)
                )
            # multi-hot: the edge whose unique src sits at logical position
            # i adds its multiplicity at (partition i%128, chunk i//128,
            # col = local dst)
            iq = slot_to_i[inv]
            c = iq // 128
            p = iq % 128
            np.add.at(ohs, (p, (cofs[b] + c) * 128 + dl), mlt)
        degp = np.concatenate(
            [deg[i * NP : (i + 1) * NP], np.ones(nblk * BLK - NP, np.float32)]
        )
        per_core.append(
            {
                "x_tr": np.ascontiguousarray(x[i * NP : (i + 1) * NP].T),
                "deg_own": np.ascontiguousarray(degp.reshape(nblk, BLK).T),
                "deg_row": np.ascontiguousarray(degp.reshape(1, nblk * BLK)),
                "gidx": gidx,
                "ohs": ohs.astype(ml_dtypes.float8_e4m3),
            }
        )

    meta = {
        "seg_valid": seg_valid,
        "N": N,
        "NP": NP,
        "IN": IN,
        "nblk": nblk,
        "CH": [int(c) for c in CH],
        "cofs": [int(c) for c in cofs],
        "widths": widths,
        "NCHT": NCHT,
    }
    return per_core, meta


def build_nc(meta, HID, OUT, ncores=NCORES):
    N, NP, IN = meta["N"], meta["NP"], meta["IN"]
    nblk, CH, cofs, widths = meta["nblk"], meta["CH"], meta["cofs"], meta["widths"]
    seg_valid = meta["seg_valid"]
    NCHT = meta["NCHT"]
    KC = IN // 128
    assert IN % 128 == 0 and HID == 128 and OUT <= 512

    nc = bacc.Bacc(
        "TRN2",
        target_bir_lowering=False,
        debug=False,
        num_devices=ncores,
        num_swdge_queues=NQUEUES,
    )

    x_tr = nc.dram_tensor("x_tr", [IN, NP], F32, kind="ExternalInput")
    w1 = nc.dram_tensor("w1", [IN, HID], F32, kind="ExternalInput")
    b1c = nc.dram_tensor("b1c", [HID, 1], F32, kind="ExternalInput")
    w2 = nc.dram_tensor("w2", [HID, OUT], F32, kind="ExternalInput")
    b2 = nc.dram_tensor("b2", [1, OUT], F32, kind="ExternalInput")
    deg_own = nc.dram_tensor("deg_own", [128, nblk], F32, kind="ExternalInput")
    deg_row = nc.dram_tensor("deg_row", [1, nblk * BLK], F32, kind="ExternalInput")
    gidx_d = nc.dram_tensor("gidx", [128, NCHT * 8], I16, kind="ExternalInput")
    ident_d = nc.dram_tensor("ident", [128, 128], BF16, kind="ExternalInput")
    ohs_d = nc.dram_tensor("ohs", [128, NCHT * 128], FP8, kind="ExternalInput")
    y = nc.dram_tensor("y", [NP, OUT], F32, kind="ExternalOutput")

    # layer-2 table rows are padded to 128 cols: dma_gather requires
    # elem_size to be a multiple of 256 bytes
    hs1_stage = nc.dram_tensor("hs1_stage", [NP, HID], BF16)
    hs1_full = nc.dram_tensor("hs1_full", [N, HID], BF16, addr_space="Shared")
    hsr2_stage = nc.dram_tensor("hsr2_stage", [NP, 128], BF16)
    hsr2_full = nc.dram_tensor("hsr2_full", [N, 128], BF16, addr_space="Shared")
    rg = [list(range(ncores))]
    qn = [0]

    def next_q():
        q = qn[0]
        qn[0] = (q + 1) % NQUEUES
        return q

    with tile.TileContext(nc) as tc:
        with (
            tc.tile_pool(name="const", bufs=1) as constp,
            tc.tile_pool(name="gath", bufs=10) as gathp,
            tc.tile_pool(name="hs", bufs=4) as hsp,
            tc.tile_pool(name="ps", bufs=3, space="PSUM") as psp,
            tc.tile_pool(name="pso", bufs=2, space="PSUM") as psop,
        ):
            nc.gpsimd.load_library(library_config.mlp)

            # ---- loads needed by phase B ----
            xsb = []
            for k in range(KC):
                t = constp.tile([128, NP], F32, tag=f"x{k}")
                nc.sync.dma_start(out=t[:], in_=x_tr[k * 128 : (k + 1) * 128, :])
                xsb.append(t)
            w1c = []
            for k in range(KC):
                t = constp.tile([128, HID], F32, tag=f"w1c{k}")
                nc.sync.dma_start(out=t[:], in_=w1[k * 128 : (k + 1) * 128, :])
                w1c.append(t)
            dinv_sb = constp.tile([128, nblk], F32, tag="dinv")
            nc.sync.dma_start(out=dinv_sb[:], in_=deg_own[:, :])
            nc.scalar.sqrt(dinv_sb[:], dinv_sb[:])
            nc.vector.reciprocal(dinv_sb[:], dinv_sb[:])

            # ---- remaining constants (overlap the AllGather) ----
            gidx_sb = constp.tile([128, NCHT * 8], I16, tag="gidx")
            nc.sync.dma_start(out=gidx_sb[:], in_=gidx_d[:, :])
            ident_sb = constp.tile([128, 128], BF16, tag="ident")
            nc.sync.dma_start(out=ident_sb[:], in_=ident_d[:, :])

            # one-hot scatter matrices, fp8, SBUF-resident for both layers
            ohs_sb = constp.tile([128, NCHT * 128], FP8, tag="ohs")
            qcols = _cdiv(NCHT * 128, 4)
            for q in range(4):
                c0q = q * qcols
                c1q = min((q + 1) * qcols, NCHT * 128)
                nc.sync.dma_start(out=ohs_sb[:, c0q:c1q], in_=ohs_d[:, c0q:c1q])

            w2_sb = constp.tile([HID, OUT], F32, tag="w2")
            nc.sync.dma_start(out=w2_sb[:], in_=w2[:, :])
            b1_sb = constp.tile([HID, 1], F32, tag="b1")
            nc.sync.dma_start(out=b1_sb[:], in_=b1c[:, :])
            b2_sb = constp.tile([1, OUT], F32, tag="b2")
            nc.sync.dma_start(out=b2_sb[:], in_=b2[:, :])
            ones_sb = constp.tile([1, 128], F32, tag="ones")
            nc.vector.memset(ones_sb[:], 1.0)

            # ---- phase B: hs1 = dinv * (x @ W1) for own rows ----
            hs1_t = []
            for b in range(nblk):
                w = widths[b]
                ph = psp.tile([128, HID], F32, tag="acc")
                for k in range(KC):
                    nc.tensor.matmul(
                        ph[:w, :],
                        lhsT=xsb[k][:, b * BLK : b * BLK + w],
                        rhs=w1c[k][:, :],
                        start=(k == 0),
                        stop=(k == KC - 1),
                    )
                t = constp.tile([128, HID], BF16, tag=f"hs1_{b}")
                nc.scalar.activation(
                    t[:w, :],
                    ph[:w, :],
                    mybir.ActivationFunctionType.Copy,
                    scale=dinv_sb[:w, b : b + 1],
                )
                nc.sync.dma_start(
                    out=hs1_stage[b * BLK : b * BLK + w, :], in_=t[:w, :]
                )
                hs1_t.append(t)

            nc.gpsimd.collective_compute(
                "AllGather",
                mybir.AluOpType.bypass,
                replica_groups=rg,
                ins=[hs1_stage[0:NP, :].opt()],
                outs=[hs1_full[0 : ncores * NP, :].opt()],
            )

            # broadcast b2 to all partitions via rank-1 matmul
            pb2 = psop.tile([128, 128], F32, tag="aux")
            nc.tensor.matmul(pb2[:, :OUT], lhsT=ones_sb[:], rhs=b2_sb[:],
                             start=True, stop=True)
            b2_bc = constp.tile([128, OUT], F32, tag="b2bc")
            nc.vector.tensor_copy(b2_bc[:], pb2[:, :OUT])

            # per-column dinv for the transposed layer-1 epilogue: broadcast
            # deg down partitions per block, then one sqrt + reciprocal
            deg_rsb = constp.tile([1, nblk * BLK], F32, tag="degrow")
            nc.sync.dma_start(out=deg_rsb[:], in_=deg_row[:, :])
            dinv_bc = constp.tile([128, nblk * BLK], F32, tag="dinvbc")
            for b in range(nblk):
                pdv = psop.tile([128, 128], F32, tag="aux")
                nc.tensor.matmul(
                    pdv[:], lhsT=ones_sb[:],
                    rhs=deg_rsb[:, b * BLK : (b + 1) * BLK],
                    start=True, stop=True,
                )
                nc.vector.tensor_copy(dinv_bc[:, b * BLK : (b + 1) * BLK], pdv[:])
            nc.scalar.sqrt(dinv_bc[:], dinv_bc[:])
            nc.vector.reciprocal(dinv_bc[:], dinv_bc[:])

            # ---- phase D: S1^T -> hsr^T -> hsr2 = dinv*(hsr @ W2) ----
            segi = [0]
            hsr2_t = []
            for b in range(nblk):
                w = widths[b]
                ch = CH[b]
                c0 = cofs[b]
                p1 = psp.tile([128, 128], F32, tag="acc")
                # self-loop: S1^T += hs1[own block]^T via identity matmul
                nc.tensor.matmul(
                    p1[:, :w], lhsT=hs1_t[b][:w, :], rhs=ident_sb[:w, :w],
                    start=True, stop=False,
                )
                for s0 in range(0, ch, MAXCH):
                    sch = min(MAXCH, ch - s0)
                    nv = seg_valid[segi[0]]
                    segi[0] += 1
                    g1 = gathp.tile([128, MAXCH, HID], BF16, tag="g")
                    nc.gpsimd.dma_gather(
                        g1[:, :sch, :],
                        hs1_full.ap(),
                        gidx_sb[:, (c0 + s0) * 8 : (c0 + s0 + sch) * 8],
                        sch * 128,
                        nv,
                        HID,
                        queue_num=next_q(),
                    )
                    for c in range(sch):
                        vk = min(128, nv - c * 128)
                        nc.tensor.matmul(
                            p1[:, :w],
                            lhsT=g1[:vk, c, :],
                            rhs=ohs_sb[:vk, (c0 + s0 + c) * 128 : (c0 + s0 + c) * 128 + w],
                            start=False,
                            stop=(s0 + c == ch - 1),
                        )
                # hsr^T = relu(dinv_col * S1^T + b1); hsr2 = dinv*(hsr @ W2)
                t1 = hsp.tile([128, 128], F32, tag="t1")
                nc.vector.tensor_tensor(
                    out=t1[:, :w], in0=p1[:, :w],
                    in1=dinv_bc[:, b * BLK : b * BLK + w],
                    op=mybir.AluOpType.mult,
                )
                hsrT = hsp.tile([128, 128], F32, tag="hsrT")
                nc.scalar.activation(
                    hsrT[:, :w], t1[:, :w],
                    mybir.ActivationFunctionType.Relu,
                    bias=b1_sb[:, 0:1],
                )
                p2s = psop.tile([128, 128], F32, tag="aux")
                nc.tensor.matmul(
                    p2s[:w, :OUT], lhsT=hsrT[:, :w], rhs=w2_sb[:, :],
                    start=True, stop=True,
                )
                t2 = constp.tile([128, 128], BF16, tag=f"hsr2_{b}")
                nc.vector.memset(t2[:, OUT:], 0.0)
                nc.scalar.activation(
                    t2[:w, :OUT], p2s[:w, :OUT],
                    mybir.ActivationFunctionType.Copy,
                    scale=dinv_sb[:w, b : b + 1],
                )
                nc.sync.dma_start(
                    out=hsr2_stage[b * BLK : b * BLK + w, :], in_=t2[:w, :]
                )
                hsr2_t.append(t2)

            nc.gpsimd.collective_compute(
                "AllGather",
                mybir.AluOpType.bypass,
                replica_groups=rg,
                ins=[hsr2_stage[0:NP, :].opt()],
                outs=[hsr2_full[0 : ncores * NP, :].opt()],
            )

            # ---- phase F: S2 -> y ----
            segi[0] = 0
            for b in range(nblk):
                w = widths[b]
                ch = CH[b]
                c0 = cofs[b]
                p2 = psp.tile([128, 128], F32, tag="acc")
                # self-loop: S2 += hsr2[own block] via identity matmul
                nc.tensor.matmul(
                    p2[:w, :OUT], lhsT=ident_sb[:w, :w], rhs=hsr2_t[b][:w, :OUT],
                    start=True, stop=False,
                )
                for s0 in range(0, ch, MAXCH):
                    sch = min(MAXCH, ch - s0)
                    nv = seg_valid[segi[0]]
                    segi[0] += 1
                    g2 = gathp.tile([128, MAXCH, 128], BF16, tag="g")
                    nc.gpsimd.dma_gather(
                        g2[:, :sch, :],
                        hsr2_full.ap(),
                        gidx_sb[:, (c0 + s0) * 8 : (c0 + s0 + sch) * 8],
                        sch * 128,
                        nv,
                        128,
                        queue_num=next_q(),
                    )
                    for c in range(sch):
                        vk = min(128, nv - c * 128)
                        nc.tensor.matmul(
                            p2[:w, :OUT],
                            lhsT=ohs_sb[:vk, (c0 + s0 + c) * 128 : (c0 + s0 + c) * 128 + w],
                            rhs=g2[:vk, c, :OUT],
                            start=False,
                            stop=(s0 + c == ch - 1),
                        )
                # y = dinv*S2 + b2
                o1 = hsp.tile([128, OUT], F32, tag="o1")
                nc.scalar.activation(
                    o1[:w, :], p2[:w, :OUT],
                    mybir.ActivationFunctionType.Copy,
                    scale=dinv_sb[:w, b : b + 1],
                )
                yt = hsp.tile([128, OUT], F32, tag="yt")
                nc.vector.tensor_tensor(
                    out=yt[:w, :], in0=o1[:w, :], in1=b2_bc[:w, :],
                    op=mybir.AluOpType.add,
                )
                nc.sync.dma_start(out=y[b * BLK : b * BLK + w, :], in_=yt[:w, :])

    nc.compile()
    return nc


def _make_ident():
    import ml_dtypes

    return np.eye(128, dtype=np.float32).astype(ml_dtypes.bfloat16)


_IDENT = _make_ident()


def make_in_maps(per_core, W1, b1, W2, b2):
    W1 = np.ascontiguousarray(np.asarray(W1, np.float32))
    W2 = np.ascontiguousarray(np.asarray(W2, np.float32))
    b1 = np.ascontiguousarray(np.asarray(b1, np.float32).reshape(-1, 1))
    b2 = np.asarray(b2, np.float32).reshape(1, -1)
    return [
        {
            "x_tr": pc["x_tr"],
            "w1": W1,
            "b1c": b1,
            "w2": W2,
            "b2": b2,
            "deg_own": pc["deg_own"],
            "deg_row": pc["deg_row"],
            "gidx": pc["gidx"],
            "ohs": pc["ohs"],
            "ident": _IDENT,
        }
        for pc in per_core
    ]


def kernel_run(x, edge_index, W1, b1, W2, b2, trace=False):
    x = np.ascontiguousarray(np.asarray(x, np.float32))
    per_core, meta = preprocess(x, edge_index)
    HID = np.asarray(W1).shape[1]
    OUT = np.asarray(W2).shape[1]
    nc = build_nc(meta, HID, OUT)
    in_maps = make_in_maps(per_core, W1, b1, W2, b2)
    res = run_bass_kernel_spmd(nc, in_maps, core_ids=list(range(NCORES)), trace=trace)
    out = np.concatenate([r["y"] for r in res.results], axis=0)
    return out, res


def kernel(x, edge_index, W1, b1, W2, b2):
    out, _ = kernel_run(x, edge_index, W1, b1, W2, b2)
    return out
